# revision 23
# baseline (speedup 1.0000x reference)
"""GNN encoder kernel for Trainium2, 8 NeuronCores, edge-sharded by sorted dst.

Feature-major activations [feat, batch], fp16 matmuls, fp32 accumulation.
  NodeEncoder(features) -> x  (its LayerNorm is folded into EdgeProcessor L1
      via column-scale pushing: relu(r*z) = r*relu(z) for r>0, and LN is
      scale-invariant, so only -mu (rank-1) and std (ea pre-scale) are needed)
  EdgeEncoder(dists) -> ea    (LN over 2 channels via folded difference matrix)
  EdgeProcessor -> ea2 (residual), scatter-added into agg via one-hot matmuls
  NodeProcessor(agg) -> h3 output rows (x==0 there: only agg rows of its W1)

Sharding: edges sorted by dst; each core owns a contiguous range of 128-node
blocks; 512-edge chunks are packed so each chunk's dsts fit one 256-node
window. The window offset is per-core data (register-driven flush-add), so a
single SPMD instruction stream serves all cores.
"""
import numpy as np

import concourse.bass as bass
import concourse.tile as tile
from concourse import mybir
from concourse.bass_utils import run_bass_kernel_spmd
from concourse.masks import make_identity

F16 = mybir.dt.float16
F32 = mybir.dt.float32
I32 = mybir.dt.int32
F8 = mybir.dt.float8e4
F8NP = mybir.dt.np(F8)

NUM_LATLONS = 259200
NUM_H3 = 41162
D = 256
EPS = 1e-5
NCORES = 8
P = 128
CHUNK = 512
TPC = CHUNK // P
GC = 2048           # scatter-group chunk (4 sub-chunks)
WIN = 640


# --- Patch: this toolchain's walrus allows only ONE sync wait per engine
# instruction. Tile's tail drain carries ~12; split into chained drains.
def _patched_drain_and_barrier(self, tick_clock, wait_clock):
    from concourse.tile import ScopedClock

    drain_inst = self.nc.sync.drain()
    wait_clock.add_sem_waits(
        drain_inst.ins, ScopedClock({None: tick_clock.global_clock})
    )
    ins = drain_inst.ins
    si = ins.sync_info
    if si is not None and len(si.on_wait) > 1:
        waits = list(si.on_wait)
        ins.sync_info = mybir.SyncInfo(on_wait=waits[:1], on_update=list(si.on_update))
        for w in waits[1:]:
            d2 = self.nc.sync.drain()
            d2.ins.sync_info = mybir.SyncInfo(on_wait=[w], on_update=[])
    self.nc.all_engine_barrier()
    assert self.sems is not None
    popped = self.nc._tile_sem_poison_stack.pop()
    assert popped is self._sem_poison
    self.nc.clear_and_free_semaphores(list(self.sems.allocated().values()))
    self.nc.all_engine_barrier()


tile.TileContext._drain_and_barrier = _patched_drain_and_barrier


def _split_waits(nc):
    """Walrus (external neuronxcc) embeds at most ONE sync wait per engine
    instruction. Rebuild blocks, moving extra waits onto single-wait NoOps
    inserted immediately before the offending instruction (same engine)."""
    m = nc.m
    n = 0
    newfs = []
    for f in m.functions:
        newblocks = []
        for blk in f.blocks:
            newinsts = []
            for inst in blk.instructions:
                si = inst.sync_info
                if si is not None and len(si.on_wait) > 1:
                    waits = list(si.on_wait)
                    for w in waits[:-1]:
                        nop = mybir.InstNoOp(
                            name=f"I-wsplit-{n}", engine=inst.engine,
                            ins=[], outs=[],
                            sync_info=mybir.SyncInfo(on_wait=[w], on_update=[]))
                        n += 1
                        newinsts.append(nop)
                    inst.sync_info = mybir.SyncInfo(
                        on_wait=[waits[-1]], on_update=list(si.on_update))
                newinsts.append(inst)
            newblocks.append(mybir.BasicBlock(
                name=blk.name, instructions=newinsts,
                IsPredicated=blk.IsPredicated, IsExit=blk.IsExit,
                IsLoopEntry=blk.IsLoopEntry))
        newfs.append(mybir.Function(
            name=f.name, attributes=f.attributes, blocks=newblocks,
            allocations=f.allocations, bass_discard=f.bass_discard))
    m.functions = newfs
    return n


def check_wait_limits(nc):
    bad = []
    for f in nc.m.functions:
        for blk in f.blocks:
            for inst in blk.instructions:
                si = inst.sync_info
                if si is not None and len(si.on_wait) > 1:
                    bad.append((inst.name, type(inst).__name__, str(inst.engine),
                                [w.ant_name for w in si.on_wait]))
    return bad


# ----------------------------------------------------------------- host prep
def _prep(features, h3_distances, edge_targets, ne, ee, ep, npr):
    f32, f16 = np.float32, np.float16
    dst = np.asarray(edge_targets).astype(np.int64) - NUM_LATLONS
    E = dst.shape[0]
    order = np.argsort(dst, kind="stable")
    dsts = dst[order]

    nblocks = (NUM_H3 + P - 1) // P
    blk_edge_start = np.searchsorted(dsts, np.arange(nblocks + 1) * P)
    bounds = [0]
    for c in range(1, NCORES):
        target = c * E / NCORES
        b = int(np.argmin(np.abs(blk_edge_start - target)))
        b = min(max(b, bounds[-1] + 1), nblocks - (NCORES - c))
        bounds.append(b)
    bounds.append(nblocks)

    core_chunks = []
    nblkl = 0
    for c in range(NCORES):
        b0, b1 = bounds[c], bounds[c + 1]
        nblkl = max(nblkl, b1 - b0)
        e0, e1 = int(blk_edge_start[b0]), int(blk_edge_start[b1])
        chunks = []
        p = e0
        while p < e1:
            wb = int(dsts[p]) // P
            q = int(np.searchsorted(dsts, (wb + WIN // P) * P))
            take = min(GC, min(q, e1) - p)
            chunks.append((p, take, wb - b0))
            p += take
        core_chunks.append(chunks)
    nchunks = max(len(ch) for ch in core_chunks)
    e_pad = nchunks * GC
    nh3pad = nblkl * P
    aggw = (nblkl + 2) * P

    featsrc = np.asarray(features, dtype=f32)
    distsrc = np.asarray(h3_distances, dtype=f32)

    def unp(prm):
        Ws, bs, gamma, beta = prm
        return ([np.asarray(w, f32) for w in Ws], [np.asarray(b, f32) for b in bs],
                np.asarray(gamma, f32), np.asarray(beta, f32))

    neW, neB, neG, neBe = unp(ne)
    eeW, eeB, eeG, eeBe = unp(ee)
    epW, epB, epG, epBe = unp(ep)
    npW, npB, npG, npBe = unp(npr)
    allb = neB + eeB + epB + npB + [epBe, eeBe, neBe]
    assert all(not np.any(b) for b in allb), "nonzero biases unsupported in fast path"

    D2e = np.array([[0.5, -0.5], [-0.5, 0.5]], f32) @ np.diag(eeG)
    D2p = np.array([[0.5, -0.5], [-0.5, 0.5]], f32) @ np.diag(epG)
    epWa = np.diag(neG) @ epW[0][0:256, :]
    epWb = epW[0][512:514, :]
    cvec = epWa.sum(axis=0)
    npW1a = npW[0][256:258, :]

    def pk(w):
        # [256, n] -> [128, 2*n]: K-half k occupies columns [k*n, (k+1)*n)
        n = w.shape[1]
        return np.concatenate([w[0:128, :], w[128:256, :]], axis=1).astype(f16)

    wdict = {
        "neW1": neW[0].astype(f16), "neW2": pk(neW[1]), "neW3": pk(neW[2]),
        "eeW1": eeW[0].astype(f16), "eeW2": pk(eeW[1]),
        "eeW3d": pk(eeW[2] @ D2e),
        "epWa": pk(epWa), "epWb": epWb.astype(f16),
        "epW2": pk(epW[1]), "epW3d": pk(epW[2] @ D2p),
        "npW1a": npW1a.astype(f16), "npW2": pk(npW[1]), "npW3": pk(npW[2]),
        "cvec": cvec.astype(f16).reshape(1, 256),
        "gnp": npG.astype(f16).reshape(1, 256),
        "e12": np.concatenate([
            np.tile(np.array([[-1.0 / 256, 0.0]], f32), (P, 1)),
            np.tile(np.array([[0.0, 1.0 / 256]], f32), (P, 1))], axis=1).astype(f16),
    }
    npbeta = npBe.astype(f32)

    gtpc = GC // P
    iota_w = np.arange(WIN)
    in_maps, meta = [], []
    for c in range(NCORES):
        chunks = core_chunks[c]
        featT = np.zeros((78, e_pad), f16)
        distT = np.zeros((2, e_pad), f16)
        onehot = np.zeros((nchunks, gtpc, P, WIN), f16)
        colmeta = np.zeros((1, nchunks), np.int32)
        for ci, (p0, take, wrel) in enumerate(chunks):
            sel = order[p0:p0 + take]
            base = ci * GC
            featT[:, base:base + take] = featsrc[sel].T
            distT[:, base:base + take] = distsrc[sel].T
            cols = dsts[p0:p0 + take] - (wrel + bounds[c]) * P
            assert cols.min() >= 0 and cols.max() < WIN
            for t in range(gtpc):
                lo, hi = t * P, min((t + 1) * P, take)
                if lo >= hi:
                    break
                onehot[ci, t, 0:hi - lo, :] = (iota_w[None, :] == cols[lo:hi, None])
            colmeta[0, ci] = wrel * P
        m = {"featT": featT, "distT": distT,
             "onehot": np.ascontiguousarray(onehot.transpose(0, 2, 1, 3)).reshape(
                 nchunks, P, gtpc * WIN).astype(F8NP),
             "colmeta": colmeta}
        m.update(wdict)
        in_maps.append(m)
        meta.append((bounds[c], bounds[c + 1]))
    return in_maps, meta, nchunks, nblkl, nh3pad, aggw, npbeta


# ------------------------------------------------------------- device program
def _build(nchunks, nblkl, nh3pad, aggw):
    nc = bass.Bass()
    e_pad = nchunks * GC
    dp = nc.declare_dram_parameter
    featT_e = dp("featT", [78, e_pad], F16, isOutput=False)
    distT_e = dp("distT", [2, e_pad], F16, isOutput=False)
    onehot_e = dp("onehot", [nchunks, P, (GC // P) * WIN], F8, isOutput=False)
    colmeta_e = dp("colmeta", [1, nchunks], I32, isOutput=False)
    w_e = {}
    for nm, shp in [("neW1", [78, 256]), ("neW2", [128, 512]), ("neW3", [128, 512]),
                    ("eeW1", [2, 256]), ("eeW2", [128, 512]), ("eeW3d", [128, 4]),
                    ("epWa", [128, 512]), ("epWb", [2, 256]), ("epW2", [128, 512]),
                    ("epW3d", [128, 4]), ("npW1a", [2, 256]), ("npW2", [128, 512]),
                    ("npW3", [128, 512]), ("cvec", [1, 256]), ("gnp", [1, 256]),
                    ("e12", [P, 4])]:
        w_e[nm] = dp(nm, shp, F16, isOutput=False)
    out_e = dp("outT", [256, nh3pad], F32, isOutput=True)

    RELU = mybir.ActivationFunctionType.Relu
    SQRT = mybir.ActivationFunctionType.Sqrt
    SQ = mybir.ActivationFunctionType.Square
    MUL = mybir.AluOpType.mult
    ADD = mybir.AluOpType.add
    SUB = mybir.AluOpType.subtract
    EQ = mybir.AluOpType.is_equal

    with tile.TileContext(nc) as tc:
        with (
            tc.tile_pool(name="const", bufs=1) as cp,
            tc.tile_pool(name="acts", bufs=3) as ap,
            tc.tile_pool(name="ohp", bufs=2) as ohp,
            tc.tile_pool(name="sml", bufs=3) as sp,
            tc.tile_pool(name="agg", bufs=1) as gp,
            tc.tile_pool(name="ps2", bufs=2, space="PSUM") as ps2,
            tc.tile_pool(name="pst", bufs=2, space="PSUM") as pst,
            tc.tile_pool(name="psa", bufs=1, space="PSUM") as psa,
        ):
            W = {}
            for nm in w_e:
                shp = list(w_e[nm].shape)
                t = cp.tile(shp, F16, tag="w_" + nm)
                nc.sync.dma_start(out=t[:], in_=w_e[nm][:])
                W[nm] = t
            ident = cp.tile([P, P], F16, tag="ident")
            make_identity(nc, ident[:])
            epsc = cp.tile([P, 1], F32, tag="epsc")
            nc.vector.memset(epsc[:], EPS)
            colmeta_sb = cp.tile([1, nchunks], I32, tag="colmeta")
            nc.sync.dma_start(out=colmeta_sb[:], in_=colmeta_e[:])
            aggT = gp.tile([2, aggw], F32, tag="aggT")
            nc.vector.memset(aggT[:], 0.0)
            # absorb const DMA waits on their consuming engines
            tch = cp.tile([1, 8], F16, tag="tch")
            nc.vector.tensor_copy(out=tch[0:1, 0:2], in_=W["e12"][0:1, 0:2])
            nc.vector.tensor_copy(out=tch[0:1, 4:6], in_=W["cvec"][0:1, 0:2])
            nc.vector.tensor_copy(out=tch[0:1, 6:8], in_=W["gnp"][0:1, 0:2])

            def mlayer(psA, psB, lhs_list, n):
                for mh, pstile in ((0, psA), (1, psB)):
                    first = True
                    for lhsT, rk in lhs_list:
                        nc.tensor.matmul(
                            pstile[:, 0:n], lhsT=lhsT[:, mh * P:(mh + 1) * P],
                            rhs=rk, start=first, stop=False)
                        first = False

            wreg = nc.vector.alloc_register("wreg")
            wv = nc.vector.snap(wreg, donate=True, min_val=0, max_val=aggw - WIN)
            wreg2 = nc.gpsimd.alloc_register("wreg2")
            wv2 = nc.gpsimd.snap(wreg2, donate=True, min_val=0, max_val=aggw - WIN)

            SUBS = GC // CHUNK
            for gci in range(nchunks):
              aggp = psa.tile([2, WIN], F32, tag="aggp")
              ohc = ohp.tile([P, (GC // P), WIN], F16, tag="ohc")
              nc.gpsimd.dma_start(out=ohc[:], in_=onehot_e[gci])
              for sub in range(SUBS):
                ci = gci * SUBS + sub
                cb = ci * CHUNK
                featc = ap.tile([78, CHUNK], F16, tag="featc")
                nc.sync.dma_start(out=featc[:], in_=featT_e[:, cb:cb + CHUNK])
                distc = ap.tile([2, CHUNK], F16, tag="distc")
                nc.sync.dma_start(out=distc[:], in_=distT_e[:, cb:cb + CHUNK])

                # ---------------- NodeEncoder
                ne1a = ps2.tile([P, CHUNK], F32, tag="mmA")
                ne1b = ps2.tile([P, CHUNK], F32, tag="mmB")
                mlayer(ne1a, ne1b, [(W["neW1"], featc[:])], CHUNK)
                h1a = ap.tile([P, CHUNK], F16, tag="ne_h1a")
                h1b = ap.tile([P, CHUNK], F16, tag="ne_h1b")
                nc.scalar.activation(out=h1a[:], in_=ne1a[:], func=RELU, scale=1.0)
                nc.scalar.activation(out=h1b[:], in_=ne1b[:], func=RELU, scale=1.0)

                ne2a = ps2.tile([P, CHUNK], F32, tag="mmA")
                ne2b = ps2.tile([P, CHUNK], F32, tag="mmB")
                mlayer(ne2a, ne2b, [(W["neW2"][:, 0:256], h1a[:]), (W["neW2"][:, 256:512], h1b[:])], CHUNK)
                h2a = ap.tile([P, CHUNK], F16, tag="ne_h2a")
                h2b = ap.tile([P, CHUNK], F16, tag="ne_h2b")
                nc.scalar.activation(out=h2a[:], in_=ne2a[:], func=RELU, scale=1.0)
                nc.scalar.activation(out=h2b[:], in_=ne2b[:], func=RELU, scale=1.0)

                ne3a = ps2.tile([P, CHUNK], F32, tag="mmA")
                ne3b = ps2.tile([P, CHUNK], F32, tag="mmB")
                mlayer(ne3a, ne3b, [(W["neW3"][:, 0:256], h2a[:]), (W["neW3"][:, 256:512], h2b[:])], CHUNK)
                hxa = ap.tile([P, CHUNK], F16, tag="ne_hxa")
                hxb = ap.tile([P, CHUNK], F16, tag="ne_hxb")
                nc.vector.tensor_copy(out=hxa[:], in_=ne3a[:])
                nc.vector.tensor_copy(out=hxb[:], in_=ne3b[:])
                hsqa = ap.tile([P, CHUNK], F16, tag="ne_h1a")
                hsqb = ap.tile([P, CHUNK], F16, tag="ne_h1b")
                nc.scalar.activation(out=hsqa[:], in_=hxa[:], func=SQ, scale=1.0)
                nc.scalar.activation(out=hsqb[:], in_=hxb[:], func=SQ, scale=1.0)

                statp0 = pst.tile([1, CHUNK], F32, tag="tt")
                statp1 = pst.tile([1, CHUNK], F32, tag="tt")
                nc.tensor.matmul(statp0[:], lhsT=W["e12"][:, 0:1], rhs=hxa[:], start=True, stop=False)
                nc.tensor.matmul(statp0[:], lhsT=W["e12"][:, 0:1], rhs=hxb[:], start=False, stop=True)
                nc.tensor.matmul(statp1[:], lhsT=W["e12"][:, 3:4], rhs=hsqa[:], start=True, stop=False)
                nc.tensor.matmul(statp1[:], lhsT=W["e12"][:, 3:4], rhs=hsqb[:], start=False, stop=True)
                s0_sb = sp.tile([1, CHUNK], F32, tag="stat_sb")
                nc.vector.tensor_copy(out=s0_sb[:], in_=statp0[:])
                musq = sp.tile([1, CHUNK], F32, tag="musq")
                nc.vector.tensor_tensor(out=musq[:], in0=s0_sb[:], in1=s0_sb[:], op=MUL)
                varr = sp.tile([1, CHUNK], F32, tag="varr")
                nc.vector.tensor_tensor(out=varr[:], in0=statp1[:], in1=musq[:], op=SUB)
                mu16 = sp.tile([1, CHUNK], F16, tag="mu16")      # -mu
                nc.vector.tensor_copy(out=mu16[:], in_=s0_sb[:])
                std16 = sp.tile([1, CHUNK], F16, tag="std16")
                nc.scalar.activation(out=std16[:], in_=varr[:], func=SQRT,
                                     bias=epsc[0:1], scale=1.0)
                stdbm_p = pst.tile([P, 2 * TPC], F16, tag="tt")
                for t in range(TPC):
                    nc.tensor.transpose(out=stdbm_p[:, 2 * t:2 * t + 1],
                                        in_=std16[0:1, t * P:(t + 1) * P],
                                        identity=ident[0:1, 0:1])
                stdbm = sp.tile([P, TPC], F32, tag="stdbm")
                nc.vector.tensor_copy(
                    out=stdbm[:], in_=stdbm_p[:].rearrange("p (t two) -> p t two", two=2)[:, :, 0])
                rstdbm = sp.tile([P, TPC], F32, tag="rstdbm")
                nc.vector.reciprocal(out=rstdbm[:], in_=stdbm[:])

                # ---------------- EdgeEncoder
                ee1a = ps2.tile([P, CHUNK], F32, tag="mmA")
                ee1b = ps2.tile([P, CHUNK], F32, tag="mmB")
                mlayer(ee1a, ee1b, [(W["eeW1"], distc[:])], CHUNK)
                g1a = ap.tile([P, CHUNK], F16, tag="ee_g1a")
                g1b = ap.tile([P, CHUNK], F16, tag="ee_g1b")
                nc.scalar.activation(out=g1a[:], in_=ee1a[:], func=RELU, scale=1.0)
                nc.scalar.activation(out=g1b[:], in_=ee1b[:], func=RELU, scale=1.0)
                ee2a = ps2.tile([P, CHUNK], F32, tag="mmA")
                ee2b = ps2.tile([P, CHUNK], F32, tag="mmB")
                mlayer(ee2a, ee2b, [(W["eeW2"][:, 0:256], g1a[:]), (W["eeW2"][:, 256:512], g1b[:])], CHUNK)
                g2a = ap.tile([P, CHUNK], F16, tag="ee_g2a")
                g2b = ap.tile([P, CHUNK], F16, tag="ee_g2b")
                nc.scalar.activation(out=g2a[:], in_=ee2a[:], func=RELU, scale=1.0)
                nc.scalar.activation(out=g2b[:], in_=ee2b[:], func=RELU, scale=1.0)
                tpm = pst.tile([2, CHUNK], F32, tag="tt")
                nc.tensor.matmul(tpm[:], lhsT=W["eeW3d"][:, 0:2], rhs=g2a[:], start=True, stop=False)
                nc.tensor.matmul(tpm[:], lhsT=W["eeW3d"][:, 2:4], rhs=g2b[:], start=False, stop=True)
                tpm16 = sp.tile([2, CHUNK], F16, tag="tpm16")
                nc.vector.tensor_copy(out=tpm16[:], in_=tpm[:])
                tbm_p = pst.tile([P, 2 * TPC], F16, tag="tt")
                for t in range(TPC):
                    nc.tensor.transpose(out=tbm_p[:, 2 * t:2 * t + 2],
                                        in_=tpm16[:, t * P:(t + 1) * P],
                                        identity=ident[0:2, 0:2])
                tbm_s = sp.tile([P, 2 * TPC], F16, tag="tbm_s")
                nc.vector.tensor_copy(out=tbm_s[:], in_=tbm_p[:])
                tsq = sp.tile([P, 2 * TPC], F32, tag="tsq")
                nc.vector.tensor_tensor(out=tsq[:], in0=tbm_s[:], in1=tbm_s[:], op=MUL)
                rstd = sp.tile([P, 2 * TPC], F32, tag="rstd")
                nc.scalar.activation(out=rstd[:], in_=tsq[:], func=SQRT, bias=epsc[:], scale=1.0)
                nc.vector.reciprocal(out=rstd[:], in_=rstd[:])
                eabm = sp.tile([P, 2 * TPC], F16, tag="eabm")
                nc.vector.tensor_tensor(out=eabm[:], in0=tbm_s[:], in1=rstd[:], op=MUL)

                eas = sp.tile([P, 2 * TPC], F16, tag="eas")
                for t in range(TPC):
                    nc.vector.tensor_scalar(
                        out=eas[:, 2 * t:2 * t + 2], in0=eabm[:, 2 * t:2 * t + 2],
                        scalar1=stdbm[:, t:t + 1], scalar2=None, op0=MUL)
                eaT_p = pst.tile([2, CHUNK], F16, tag="tt")
                for t in range(TPC):
                    nc.tensor.transpose(out=eaT_p[:, t * P:(t + 1) * P],
                                        in_=eas[:, 2 * t:2 * t + 2], identity=ident[:])
                eaT = sp.tile([2, CHUNK], F16, tag="eaT")
                nc.vector.tensor_copy(out=eaT[:], in_=eaT_p[:])

                # ---------------- EdgeProcessor
                ep1a = ps2.tile([P, CHUNK], F32, tag="mmA")
                ep1b = ps2.tile([P, CHUNK], F32, tag="mmB")
                mlayer(ep1a, ep1b, [(W["epWa"][:, 0:256], hxa[:]), (W["epWa"][:, 256:512], hxb[:]),
                                    (W["epWb"], eaT[:])], CHUNK)
                nc.tensor.matmul(ep1a[:], lhsT=W["cvec"][:, 0:P], rhs=mu16[:], start=False, stop=True)
                nc.tensor.matmul(ep1b[:], lhsT=W["cvec"][:, P:2 * P], rhs=mu16[:], start=False, stop=True)
                q1a = ap.tile([P, CHUNK], F16, tag="ee_g1a")
                q1b = ap.tile([P, CHUNK], F16, tag="ee_g1b")
                nc.scalar.activation(out=q1a[:], in_=ep1a[:], func=RELU, scale=1.0)
                nc.scalar.activation(out=q1b[:], in_=ep1b[:], func=RELU, scale=1.0)
                ep2a = ps2.tile([P, CHUNK], F32, tag="mmA")
                ep2b = ps2.tile([P, CHUNK], F32, tag="mmB")
                mlayer(ep2a, ep2b, [(W["epW2"][:, 0:256], q1a[:]), (W["epW2"][:, 256:512], q1b[:])], CHUNK)
                q2a = ap.tile([P, CHUNK], F16, tag="ne_h2a")
                q2b = ap.tile([P, CHUNK], F16, tag="ne_h2b")
                nc.scalar.activation(out=q2a[:], in_=ep2a[:], func=RELU, scale=1.0)
                nc.scalar.activation(out=q2b[:], in_=ep2b[:], func=RELU, scale=1.0)
                ypm = pst.tile([2, CHUNK], F32, tag="tt")
                nc.tensor.matmul(ypm[:], lhsT=W["epW3d"][:, 0:2], rhs=q2a[:], start=True, stop=False)
                nc.tensor.matmul(ypm[:], lhsT=W["epW3d"][:, 2:4], rhs=q2b[:], start=False, stop=True)
                ypm16 = sp.tile([2, CHUNK], F16, tag="ypm16")
                nc.vector.tensor_copy(out=ypm16[:], in_=ypm[:])
                ybm_p = pst.tile([P, 2 * TPC], F16, tag="tt")
                for t in range(TPC):
                    nc.tensor.transpose(out=ybm_p[:, 2 * t:2 * t + 2],
                                        in_=ypm16[:, t * P:(t + 1) * P],
                                        identity=ident[0:2, 0:2])
                zbm = sp.tile([P, 2 * TPC], F32, tag="zbm")
                for t in range(TPC):
                    nc.vector.tensor_scalar(
                        out=zbm[:, 2 * t:2 * t + 2], in0=ybm_p[:, 2 * t:2 * t + 2],
                        scalar1=rstdbm[:, t:t + 1], scalar2=None, op0=MUL)
                ysq = sp.tile([P, 2 * TPC], F32, tag="tsq")
                nc.vector.tensor_tensor(out=ysq[:], in0=zbm[:], in1=zbm[:], op=MUL)
                yrstd = sp.tile([P, 2 * TPC], F32, tag="rstd")
                nc.scalar.activation(out=yrstd[:], in_=ysq[:], func=SQRT, bias=epsc[:], scale=1.0)
                nc.vector.reciprocal(out=yrstd[:], in_=yrstd[:])
                lnout = sp.tile([P, 2 * TPC], F32, tag="lnout")
                nc.vector.tensor_tensor(out=lnout[:], in0=zbm[:], in1=yrstd[:], op=MUL)
                ea2 = sp.tile([P, 2 * TPC], F16, tag="ea2")
                nc.vector.tensor_tensor(out=ea2[:], in0=lnout[:], in1=eabm[:], op=ADD)

                # ---------------- scatter (accumulate across the whole gchunk)
                for t in range(TPC):
                    gt = sub * TPC + t
                    for half, (w0, w1) in enumerate(((0, 512), (512, WIN))):
                        nc.tensor.matmul(aggp[:, w0:w1], lhsT=ea2[:, 2 * t:2 * t + 2],
                                         rhs=ohc[:, gt, w0:w1], start=(gt == 0),
                                         stop=(gt == GC // P - 1))
              if gci % 2 == 0:
                  nc.vector.load(wreg, colmeta_sb[0:1, gci:gci + 1])
                  nc.vector.tensor_tensor(
                      out=aggT[:, bass.ds(wv, WIN)], in0=aggT[:, bass.ds(wv, WIN)],
                      in1=aggp[:], op=ADD)
              else:
                  aggsb = sp.tile([2, WIN], F32, tag="aggsb")
                  nc.vector.tensor_copy(out=aggsb[:], in_=aggp[:])
                  nc.gpsimd.load(wreg2, colmeta_sb[0:1, gci:gci + 1])
                  nc.gpsimd.tensor_tensor(
                      out=aggT[:, bass.ds(wv2, WIN)], in0=aggT[:, bass.ds(wv2, WIN)],
                      in1=aggsb[:], op=ADD)

            # ---------------- NodeProcessor
            agg16 = gp.tile([2, nh3pad], F16, tag="agg16")
            nc.vector.tensor_copy(out=agg16[:], in_=aggT[:, 0:nh3pad])
            spans = [(i * CHUNK, min(CHUNK, nh3pad - i * CHUNK))
                     for i in range((nh3pad + CHUNK - 1) // CHUNK)]
            for (s0, n) in spans:
                p1a = ps2.tile([P, CHUNK], F32, tag="mmA")
                p1b = ps2.tile([P, CHUNK], F32, tag="mmB")
                mlayer(p1a, p1b, [(W["npW1a"], agg16[:, s0:s0 + n])], n)
                r1a = ap.tile([P, CHUNK], F16, tag="ne_h1a")
                r1b = ap.tile([P, CHUNK], F16, tag="ne_h1b")
                nc.scalar.activation(out=r1a[:, 0:n], in_=p1a[:, 0:n], func=RELU, scale=1.0)
                nc.scalar.activation(out=r1b[:, 0:n], in_=p1b[:, 0:n], func=RELU, scale=1.0)
                p2a = ps2.tile([P, CHUNK], F32, tag="mmA")
                p2b = ps2.tile([P, CHUNK], F32, tag="mmB")
                mlayer(p2a, p2b, [(W["npW2"][:, 0:256], r1a[:, 0:n]), (W["npW2"][:, 256:512], r1b[:, 0:n])], n)
                r2a = ap.tile([P, CHUNK], F16, tag="ne_h2a")
                r2b = ap.tile([P, CHUNK], F16, tag="ne_h2b")
                nc.scalar.activation(out=r2a[:, 0:n], in_=p2a[:, 0:n], func=RELU, scale=1.0)
                nc.scalar.activation(out=r2b[:, 0:n], in_=p2b[:, 0:n], func=RELU, scale=1.0)
                p3a = ps2.tile([P, CHUNK], F32, tag="mmA")
                p3b = ps2.tile([P, CHUNK], F32, tag="mmB")
                mlayer(p3a, p3b, [(W["npW3"][:, 0:256], r2a[:, 0:n]), (W["npW3"][:, 256:512], r2b[:, 0:n])], n)
                hoa = ap.tile([P, CHUNK], F16, tag="ne_hxa")
                hob = ap.tile([P, CHUNK], F16, tag="ne_hxb")
                nc.vector.tensor_copy(out=hoa[:, 0:n], in_=p3a[:, 0:n])
                nc.vector.tensor_copy(out=hob[:, 0:n], in_=p3b[:, 0:n])
                hsa = ap.tile([P, CHUNK], F16, tag="ee_g1a")
                hsb = ap.tile([P, CHUNK], F16, tag="ee_g1b")
                nc.scalar.activation(out=hsa[:, 0:n], in_=hoa[:, 0:n], func=SQ, scale=1.0)
                nc.scalar.activation(out=hsb[:, 0:n], in_=hob[:, 0:n], func=SQ, scale=1.0)
                stp0 = pst.tile([1, CHUNK], F32, tag="tt")
                stp1 = pst.tile([1, CHUNK], F32, tag="tt")
                nc.tensor.matmul(stp0[:, 0:n], lhsT=W["e12"][:, 0:1], rhs=hoa[:, 0:n], start=True, stop=False)
                nc.tensor.matmul(stp0[:, 0:n], lhsT=W["e12"][:, 0:1], rhs=hob[:, 0:n], start=False, stop=True)
                nc.tensor.matmul(stp1[:, 0:n], lhsT=W["e12"][:, 3:4], rhs=hsa[:, 0:n], start=True, stop=False)
                nc.tensor.matmul(stp1[:, 0:n], lhsT=W["e12"][:, 3:4], rhs=hsb[:, 0:n], start=False, stop=True)
                s0np = sp.tile([1, CHUNK], F32, tag="stat_sb")
                nc.vector.tensor_copy(out=s0np[:, 0:n], in_=stp0[:, 0:n])
                msq = sp.tile([1, CHUNK], F32, tag="musq")
                nc.vector.tensor_tensor(out=msq[:, 0:n], in0=s0np[:, 0:n], in1=s0np[:, 0:n], op=MUL)
                vr = sp.tile([1, CHUNK], F32, tag="varr")
                nc.vector.tensor_tensor(out=vr[:, 0:n], in0=stp1[:, 0:n], in1=msq[:, 0:n], op=SUB)
                rst = sp.tile([1, CHUNK], F32, tag="nprstd")
                nc.scalar.activation(out=rst[:, 0:n], in_=vr[:, 0:n], func=SQRT,
                                     bias=epsc[0:1], scale=1.0)
                nc.vector.reciprocal(out=rst[:, 0:n], in_=rst[:, 0:n])
                mr = sp.tile([1, CHUNK], F32, tag="npmr")
                nc.vector.tensor_tensor(out=mr[:, 0:n], in0=s0np[:, 0:n], in1=rst[:, 0:n], op=MUL)
                rst16 = sp.tile([1, CHUNK], F16, tag="nprstd16")
                nc.vector.tensor_copy(out=rst16[:, 0:n], in_=rst[:, 0:n])
                mr16 = sp.tile([1, CHUNK], F16, tag="npmr16")
                nc.vector.tensor_copy(out=mr16[:, 0:n], in_=mr[:, 0:n])
                for mh, (pp, ho) in enumerate(((p3a, hoa), (p3b, hob))):
                    bA = pst.tile([P, CHUNK], F32, tag="tt")
                    bB = pst.tile([P, CHUNK], F32, tag="tt")
                    nc.tensor.matmul(bA[:, 0:n], lhsT=W["gnp"][:, mh * P:(mh + 1) * P],
                                     rhs=rst16[:, 0:n], start=True, stop=True)
                    nc.tensor.matmul(bB[:, 0:n], lhsT=W["gnp"][:, mh * P:(mh + 1) * P],
                                     rhs=mr16[:, 0:n], start=True, stop=True)
                    tmp = ap.tile([P, CHUNK], F32, tag="np_tmp")
                    nc.vector.tensor_tensor(out=tmp[:, 0:n], in0=ho[:, 0:n],
                                            in1=bA[:, 0:n], op=MUL)
                    outc = ap.tile([P, CHUNK], F32, tag="np_out")
                    nc.vector.tensor_tensor(out=outc[:, 0:n], in0=tmp[:, 0:n],
                                            in1=bB[:, 0:n], op=ADD)
                    nc.sync.dma_start(out=out_e[mh * P:(mh + 1) * P, s0:s0 + n],
                                      in_=outc[:, 0:n])
    _split_waits(nc)
    return nc


_CACHE = {}


def kernel(features, h3_distances, edge_targets, ne, ee, ep, npr):
    in_maps, meta, nchunks, nblkl, nh3pad, aggw, npbeta = _prep(
        features, h3_distances, edge_targets, ne, ee, ep, npr)
    key = (nchunks, nblkl)
    if key not in _CACHE:
        _CACHE[key] = _build(nchunks, nblkl, nh3pad, aggw)
    nc = _CACHE[key]
    res = run_bass_kernel_spmd(nc, in_maps, core_ids=list(range(NCORES)))
    out = np.zeros((NUM_H3, D), np.float32)
    for c in range(NCORES):
        b0, b1 = meta[c]
        n0 = b0 * P
        n1 = min(b1 * P, NUM_H3)
        out[n0:n1] = res.results[c]["outT"][:, 0:n1 - n0].T
    if np.any(npbeta):
        out += npbeta[None, :]
    return out
